# revision 1
# baseline (speedup 1.0000x reference)
"""Multi-head attention (B=4, T=2048, D=1024, H=16 heads, causal) on 8
Trainium2 NeuronCores.

Sharding: core i handles batch b = i//2 and head-group g = i%2 (8 heads,
512 features). Each core computes its head-group's attention output and a
partial output projection; the host sums the two partials per batch.

All heavy matmuls run as float32r (fp32 storage, ~12-bit-mantissa multiply)
which streams at full PE rate for free-dim >= 256.

Attention is computed entirely in "transposed score" layout to avoid any
on-device transposes:
  ST[k, q] = K_k . Q_q   (keys on partitions, queries on free dim)
  P^T = exp(ST/8 - M0) * causal_mask   (ACT exp, per-partition bias slot)
  [OT; s] = V'^T @ P^T   where V' has a ones column, so the softmax
            denominator s comes out of the same matmul (row 64).
  OT_norm = OT * (1/s)   (DVE reciprocal + PE rank-1 broadcast of 1/s)
  y_partial = OT_norm^T @ Wo^T-slice  (natural layout, DMA'd out directly)
"""

import numpy as np

import concourse.bass as bass
import concourse.mybir as mybir
import concourse.tile as tile
from concourse import bass_utils

import orjson

D_MODEL = 1024
N_HEADS = 16
D_K = 64
B, T = 4, 2048
FG = 512            # features per head-group (8 heads x 64)
N_CORES = 8
M0 = 12.0           # global exp shift (cancels exactly in softmax)
KORDER_MODE = "diag_last"

F32R = mybir.dt.float32r
F32 = mybir.dt.float32
AF = mybir.ActivationFunctionType


# ---------------------------------------------------------------------------
# BIR legalization for the stock walrus in this container: it encodes at most
# ONE sync wait per instruction, so spill extra waits onto single-wait
# EventSemaphore nops placed immediately before the instruction on the same
# engine.  Also statically verify no wait precedes (in program order) the
# instructions that produce its semaphore value, which would deadlock.
# ---------------------------------------------------------------------------

def split_multiwaits(bir_bytes: bytes) -> bytes:
    bir = orjson.loads(bir_bytes)
    n = [0]
    for fn in bir.get("functions", []):
        for blk in fn.get("blocks", []):
            out = []
            for inst in blk.get("instructions", []):
                si = inst.get("sync_info") or {}
                waits = si.get("on_wait") or []
                if len(waits) > 1:
                    for w in waits[:-1]:
                        n[0] += 1
                        out.append({
                            "debug": inst.get("debug", 0),
                            "engine": inst["engine"],
                            "ins": [], "name": f"WSPLIT-{n[0]}",
                            "opcode": "EventSemaphore", "outs": [],
                            "sync_info": {"on_update": [], "on_wait": [w]},
                        })
                    si["on_wait"] = waits[-1:]
                    inst["sync_info"] = si
                out.append(inst)
            blk["instructions"] = out
    _check_no_forward_waits(bir)
    return orjson.dumps(bir)


def _check_no_forward_waits(bir) -> None:
    issued = {}
    for fn in bir.get("functions", []):
        for blk in fn.get("blocks", []):
            for inst in blk.get("instructions", []):
                si = inst.get("sync_info") or {}
                for w in (si.get("on_wait") or []):
                    if (w.get("sync_type") == "semaphore"
                            and w.get("wait_mode") == "sem-ge-imm"
                            and "barrier" not in (w.get("ant_name") or "")):
                        if issued.get(w["id"], 0) < w["wait_value"]:
                            raise RuntimeError(
                                f"forward wait: {inst['name']} waits "
                                f"{w.get('ant_name')}>={w['wait_value']}")
                for u in (si.get("on_update") or []):
                    if (u.get("sync_type") == "semaphore"
                            and u.get("update_mode") in ("sem-inc", "sem-add-imm")):
                        issued[u["id"]] = issued.get(u["id"], 0) + u.get("update_value", 1)


# ---------------------------------------------------------------------------
# Device program (identical on all 8 cores; data differs per core)
# ---------------------------------------------------------------------------

def build_nc(st_bufs=8, psa_bufs=4, exp_batch=1, batch_act=False, pool_mask=False, psb_bufs=2, psc_bufs=1, msc_bufs=2, act_width=512, y_from_psum=False, y_defer=False, psp_bufs=8, j_order=None, causal=True) -> bass.Bass:
    nc = bass.Bass("TRN2", target_bir_lowering=False, debug=False)

    xt = nc.dram_tensor("xt", (D_MODEL, T), F32R, kind="ExternalInput")
    wq = nc.dram_tensor("wq", (D_MODEL, FG), F32R, kind="ExternalInput")
    wk = nc.dram_tensor("wk", (D_MODEL, FG), F32R, kind="ExternalInput")
    wv = nc.dram_tensor("wv", (D_MODEL, FG), F32R, kind="ExternalInput")
    wo = nc.dram_tensor("wo", (FG, D_MODEL), F32R, kind="ExternalInput")
    cvec = nc.dram_tensor("cvec", (T,), F32, kind="ExternalInput")
    onesd = nc.dram_tensor("onesd", (1, 64), F32R, kind="ExternalInput")
    onescol = nc.dram_tensor("onescol", (128, 16), F32R, kind="ExternalInput")
    maskd = nc.dram_tensor("maskd", (128, 512), F32R, kind="ExternalInput")
    y = nc.dram_tensor("y", (T, D_MODEL), F32, kind="ExternalOutput")

    NDT = D_MODEL // 128   # 8 contraction tiles
    NFT = FG // 128        # 4 feature tiles (2 heads each)
    NTT = T // 128         # 16 token tiles
    NTS = T // 512         # 4 token strips

    with tile.TileContext(nc) as tc, nc.allow_low_precision(reason="f32r storage"):
        with (
            tc.tile_pool(name="qtp", bufs=1) as qtp,
            tc.tile_pool(name="ktp", bufs=1) as ktp,
            tc.tile_pool(name="vpp", bufs=1) as vpp,
            tc.tile_pool(name="stp", bufs=st_bufs) as stp,
            tc.tile_pool(name="cst", bufs=1) as cst,
            tc.tile_pool(name="msc", bufs=msc_bufs) as msc,
        ):
            # persistent tiles
            qt = [qtp.tile([128, T], F32R, name=f"qt{i}") for i in range(NFT)]
            kt = [ktp.tile([128, T], F32R, name=f"kt{i}") for i in range(NFT)]
            vp = [vpp.tile([128, 520], F32R, name=f"vp{i}") for i in range(NTT)]
            cb = cst.tile([128, 16], F32, name="cb")
            ones = cst.tile([1, 64], F32R, name="ones")
            mask = cst.tile([128, 512], F32R, name="mask")
            nc.sync.dma_start(cb[:], cvec.rearrange("(a b) -> b a", b=128))
            nc.sync.dma_start(ones[:], onesd[:])
            nc.sync.dma_start(mask[:], maskd[:])

            # ---- phase 1: projections ------------------------------------
            with (
                tc.tile_pool(name="xtp", bufs=1) as xtp,
                tc.tile_pool(name="wp", bufs=1) as wp,
                tc.tile_pool(name="psP", bufs=psp_bufs, space="PSUM") as psA,
            ):
                xts = [xtp.tile([128, T], F32R, name=f"xts{i}") for i in range(NDT)]
                wq_t = [wp.tile([128, FG], F32R, name=f"w{i}", tag=f"w{i}")
                        for i in range(NDT)]
                # interleave weight tiles with strip-0 x tiles so the
                # first psum accumulation can start after ~0.5MB of DMA
                for i in range(NDT):
                    nc.sync.dma_start(wq_t[i][:], wq[i * 128:(i + 1) * 128, :])
                    nc.sync.dma_start(
                        xts[i][:, 0:512], xt[i * 128:(i + 1) * 128, 0:512])
                for ts in range(1, NTS):
                    for i in range(NDT):
                        nc.sync.dma_start(
                            xts[i][:, ts * 512:(ts + 1) * 512],
                            xt[i * 128:(i + 1) * 128, ts * 512:(ts + 1) * 512])

                def emit_qk(wtl, dst):
                    for ts in range(NTS):
                        for ft in range(NFT):
                            ps = psA.tile([128, 512], F32, name="pj", tag="A")
                            for d in range(NDT):
                                nc.tensor.matmul(
                                    ps[:], wtl[d][:, ft * 128:(ft + 1) * 128],
                                    xts[d][:, ts * 512:(ts + 1) * 512],
                                    start=(d == 0), stop=(d == NDT - 1))
                            nc.vector.tensor_copy(
                                dst[ft][:, ts * 512:(ts + 1) * 512], ps[:])

                # Q^T first, then V (attention j=0 needs V'[0:4]), K^T last:
                # attention starts as soon as K^T strip 0 lands.
                emit_qk(wq_t, qt)

                wvl = [wp.tile([128, FG], F32R, name=f"wv{i}", tag=f"w{i}")
                       for i in range(NDT)]
                for i in range(NDT):
                    nc.sync.dma_start(wvl[i][:], wv[i * 128:(i + 1) * 128, :])
                for tt in range(NTT):
                    ps = psA.tile([128, 512], F32, name="pv", tag="A")
                    for d in range(NDT):
                        nc.tensor.matmul(
                            ps[:], xts[d][:, tt * 128:(tt + 1) * 128], wvl[d][:],
                            start=(d == 0), stop=(d == NDT - 1))
                    vv = vp[tt].rearrange("p (h c) -> p h c", h=8)
                    pv = ps.rearrange("p (h c) -> p h c", h=8)
                    nc.vector.tensor_copy(vv[:, :, 0:64], pv[:])
                    nc.sync.dma_start(
                        vv[:, :, 64:65],
                        onescol.rearrange("p (h c) -> p h c", h=16)[:, 0:8, :])

                wkl = [wp.tile([128, FG], F32R, name=f"wk{i}", tag=f"w{i}")
                       for i in range(NDT)]
                for i in range(NDT):
                    nc.sync.dma_start(wkl[i][:], wk[i * 128:(i + 1) * 128, :])
                emit_qk(wkl, kt)

            # ---- phase 2: attention + output projection ------------------
            with (
                tc.tile_pool(name="otp", bufs=1) as otp,
                tc.tile_pool(name="wop", bufs=1) as wop,
                tc.tile_pool(name="psA", bufs=psa_bufs, space="PSUM") as psA,
                tc.tile_pool(name="psB", bufs=psb_bufs, space="PSUM") as psB,
                tc.tile_pool(name="psC", bufs=psc_bufs, space="PSUM") as psC,
            ):
                ot = [otp.tile([128, T], F32R, name=f"ot{i}") for i in range(NFT)]
                wot = [wop.tile([128, D_MODEL], F32R, name=f"wot{i}")
                       for i in range(NFT)]
                for i in range(NFT):
                    nc.sync.dma_start(wot[i][:], wo[i * 128:(i + 1) * 128, :])

                for j in (j_order or range(NTS)):
                    for h in range(8):
                        ft, base = h // 2, (h % 2) * 64
                        if causal:
                            nkt = 4 * j + 4    # causal key tiles for strip j
                            if KORDER_MODE == "diag_first":
                                korder = list(range(4 * j, nkt)) + list(range(0, 4 * j))
                            elif KORDER_MODE == "diag_last":
                                korder = [4 * j] + list(range(0, 4 * j)) + list(range(4 * j + 1, nkt))
                            else:  # ascending
                                korder = list(range(nkt))
                        else:
                            nkt = NTT
                            korder = list(range(NTT))
                        ops = psB.tile([128, 512], F32, name="ops", tag="B")

                        def _lo(kt_):
                            if not causal:
                                return 0
                            m_ = kt_ - 4 * j
                            return 128 * m_ if m_ > 0 else 0

                        nb = act_width // 512
                        ki = 0
                        while ki < nkt:
                            gts = korder[ki:ki + nb]
                            if any(_lo(k_) for k_ in gts):
                                gts = gts[:1]
                            gn = len(gts)
                            lo0 = _lo(gts[0])
                            sps = psA.tile([128, act_width], F32, name="sps", tag="A")
                            se = stp.tile([128, act_width], F32R, name="se", tag="se")
                            for u, ktile in enumerate(gts):
                                lo = _lo(ktile)
                                nc.tensor.matmul(
                                    sps[:, u * 512 + lo:(u + 1) * 512],
                                    kt[ft][base:base + 64, ktile * 128:(ktile + 1) * 128],
                                    qt[ft][base:base + 64, j * 512 + lo:(j + 1) * 512],
                                    start=True, stop=True)
                            if gn == nb and nb > 1:
                                nc.scalar.activation(
                                    se[:, 0:gn * 512], sps[:, 0:gn * 512], AF.Exp,
                                    bias=cb[:, gts[0]:gts[0] + 1], scale=0.125)
                            else:
                                for u, ktile in enumerate(gts):
                                    lo = _lo(ktile)
                                    nc.scalar.activation(
                                        se[:, u * 512 + lo:(u + 1) * 512],
                                        sps[:, u * 512 + lo:(u + 1) * 512], AF.Exp,
                                        bias=cb[:, ktile:ktile + 1], scale=0.125)
                            for u, ktile in enumerate(gts):
                                m = (ktile - 4 * j) if causal else -1
                                if m >= 0:
                                    # only the first 128-col block of the
                                    # restricted range straddles the causal
                                    # staircase; the rest is fully unmasked
                                    lo = _lo(ktile)
                                    nc.vector.tensor_mul(
                                        se[:, u * 512 + lo:u * 512 + lo + 128],
                                        se[:, u * 512 + lo:u * 512 + lo + 128],
                                        mask[:, 0:128])
                            for u, ktile in enumerate(gts):
                                lo = _lo(ktile)
                                nc.tensor.matmul(
                                    ops[0:65, lo:512],
                                    vp[ktile][:, h * 65:(h + 1) * 65],
                                    se[:, u * 512 + lo:(u + 1) * 512],
                                    start=(ki + u == 0), stop=(ki + u == nkt - 1))
                            ki += gn
                        # normalize rows 0:64 by row 64 and store to OT
                        r = msc.tile([1, 512], F32R, name="r", tag="r")
                        nc.vector.reciprocal(r[:], ops[64:65, :])
                        rps = psC.tile([64, 512], F32, name="rps", tag="R")
                        nc.tensor.matmul(rps[:], ones[:], r[:],
                                         start=True, stop=True)
                        rb = msc.tile([64, 512], F32, name="rb", tag="rb")
                        nc.vector.tensor_copy(rb[:], rps[:])
                        nc.vector.tensor_mul(
                            ot[ft][base:base + 64, j * 512:(j + 1) * 512],
                            ops[0:64, :], rb[:])

                    # y = OT^T @ woT; optionally deferred one strip to
                    # dodge the diagonal-heavy DVE window
                    yjs = ([j - 1] if j > 0 else []) if y_defer else [j]
                    if y_defer and j == NTS - 1:
                        yjs = [j - 1, j]
                    for yj in yjs:
                      for tt in range(4 * yj, 4 * yj + 4):
                        for nn in range(2):
                            yps = psC.tile([128, 512], F32, name="yps", tag="C")
                            for ft in range(NFT):
                                nc.tensor.matmul(
                                    yps[:], ot[ft][:, tt * 128:(tt + 1) * 128],
                                    wot[ft][:, nn * 512:(nn + 1) * 512],
                                    start=(ft == 0), stop=(ft == NFT - 1))
                            ysb = msc.tile([128, 512], F32, name="ysb", tag="y")
                            nc.vector.tensor_copy(ysb[:], yps[:])
                            nc.sync.dma_start(
                                y[tt * 128:(tt + 1) * 128, nn * 512:(nn + 1) * 512],
                                ysb[:])

    _orig = nc.to_json_bytes
    nc.to_json_bytes = lambda: split_multiwaits(_orig())
    return nc


_NC = {}


def _get_nc(causal=True) -> bass.Bass:
    if causal not in _NC:
        _NC[causal] = build_nc(causal=causal)
    return _NC[causal]


# ---------------------------------------------------------------------------
# Host-side sharding + gather
# ---------------------------------------------------------------------------

def _kernel_numpy(q, mask, Wq, bq, Wk, bk, Wv, bv, Wo, bo):
    """Exact host fallback for unexpected shapes or arbitrary masks."""
    b, t, d = q.shape
    h = N_HEADS if d == D_MODEL else max(1, d // D_K)
    dk = d // h
    qh = (q @ Wq.T + bq).reshape(b, t, h, dk).transpose(0, 2, 1, 3)
    kh = (q @ Wk.T + bk).reshape(b, t, h, dk).transpose(0, 2, 1, 3)
    vh = (q @ Wv.T + bv).reshape(b, t, h, dk).transpose(0, 2, 1, 3)
    s = np.einsum("bhqd,bhkd->bhqk", qh, kh) / np.sqrt(dk).astype(np.float32)
    s = np.where(mask, -np.inf, s)
    s = s - s.max(axis=-1, keepdims=True)
    p = np.exp(s)
    p /= p.sum(axis=-1, keepdims=True)
    o = np.einsum("bhqk,bhkd->bhqd", p, vh)
    o = o.transpose(0, 2, 1, 3).reshape(b, t, d)
    return (o @ Wo.T + bo).astype(np.float32)


_CAUSAL_REF = None


def _mask_kind(mask):
    """Classify the (B,1,T,T) bool mask: "causal" / "none" / "other"."""
    global _CAUSAL_REF
    mask = np.asarray(mask)
    if mask.shape != (B, 1, T, T):
        return "other"
    if not mask.any():
        return "none"
    if _CAUSAL_REF is None:
        _CAUSAL_REF = np.triu(np.ones((T, T), dtype=bool), k=1)
    for i in range(mask.shape[0]):
        if not np.array_equal(mask[i, 0], _CAUSAL_REF):
            return "other"
    return "causal"


def kernel(q, mask, Wq, bq, Wk, bk, Wv, bv, Wo, bo):
    q = np.asarray(q, np.float32)
    Wq = np.asarray(Wq, np.float32); bq = np.asarray(bq, np.float32)
    Wk = np.asarray(Wk, np.float32); bk = np.asarray(bk, np.float32)
    Wv = np.asarray(Wv, np.float32); bv = np.asarray(bv, np.float32)
    Wo = np.asarray(Wo, np.float32); bo = np.asarray(bo, np.float32)

    kind = _mask_kind(mask)
    if q.shape != (B, T, D_MODEL) or Wq.shape != (D_MODEL, D_MODEL) or kind == "other":
        return _kernel_numpy(np.asarray(q, np.float32), np.asarray(mask, bool),
                             Wq, bq, Wk, bk, Wv, bv, Wo, bo)

    onesd = np.ones((1, 64), np.float32)
    onescol = np.ones((128, 16), np.float32)
    # causal staircase: M[k, u] = 1 iff k <= u (used as [0 : 512-128m])
    kk = np.arange(128)[:, None]
    vv_ = np.arange(512)[None, :]
    maskst = (kk <= vv_).astype(np.float32)

    in_maps = []
    for b in range(B):
        x = np.ascontiguousarray(q[b])                    # (T, D)
        xT = np.ascontiguousarray(x.T)                    # (D, T)
        # per-key exp bias: bq . K_k term (softmax-relevant) minus shift M0
        cvec = ((x @ (Wk.T @ bq) + float(bq @ bk)) * 0.125 - M0).astype(np.float32)
        for g in range(2):
            sl = slice(g * FG, (g + 1) * FG)
            in_maps.append({
                "xt": xT,
                "wq": np.ascontiguousarray(Wq[sl].T),     # (D, FG)
                "wk": np.ascontiguousarray(Wk[sl].T),
                "wv": np.ascontiguousarray(Wv[sl].T),
                "wo": np.ascontiguousarray(Wo[:, sl].T),  # (FG, D)
                "cvec": cvec,
                "onesd": onesd,
                "onescol": onescol,
                "maskd": maskst,
            })

    nc = _get_nc(causal=(kind == "causal"))
    res = bass_utils.run_bass_kernel_spmd(nc, in_maps, core_ids=list(range(N_CORES)))

    out = np.empty((B, T, D_MODEL), np.float32)
    # exact bias correction terms (zero when biases are zero):
    # V-bias contributes bv @ Wo.T (softmax rows sum to 1); plus bo.
    corr = (bv @ Wo.T + bo).astype(np.float32)
    for b in range(B):
        out[b] = res.results[2 * b]["y"] + res.results[2 * b + 1]["y"] + corr
    return out



# revision 24
# speedup vs baseline: 1.2694x; 1.2694x over previous
"""Multi-head attention (B=4, T=2048, D=1024, H=16 heads, causal) on 8
Trainium2 NeuronCores.

Sharding: core i handles batch b = i//2 and head-group g = i%2 (8 heads,
512 features). Each core computes its head-group's attention output and a
partial output projection; the host sums the two partials per batch.

v2 pipeline (all matmuls bf16, psum f32):
  - Q^T/K^T/V projections stream per 512-token strip, interleaved with
    attention so the PE never waits on the ACT-bound softmax windows.
  - Scores are computed per head-PAIR into one [128,1024] psum tile
    (head h in cols 0:512, head h+1 in 512:1024), so one exp activation
    covers two heads (160 acts instead of 320).
  - PV runs in natural layout: stationary = P^T chunk [128k,128q],
    moving = V' [128k, 65] (64 dims + ones col) -> out[128q, 65].
    65-row matmuls instead of 512-row ones (2.1x less PE time), and the
    softmax denominator s lands per-PARTITION in psum col c*65+64.
  - Normalize: one DVE reciprocal (strided s columns) + one broadcast
    multiply per head -> O_norm [128q, 512f] bf16.
  - O_norm is transposed back to feature-major via PE transposes (bf16,
    1 cyc/row) for the output projection.
"""

import numpy as np
import ml_dtypes

import concourse.bass as bass
import concourse.mybir as mybir
import concourse.tile as tile
from concourse import bass_utils

import orjson

D_MODEL = 1024
N_HEADS = 16
D_K = 64
B, T = 4, 2048
FG = 512            # features per head-group (8 heads x 64)
N_CORES = 8
M0 = 12.0           # global exp shift (cancels exactly in softmax)

F32 = mybir.dt.float32
F32R = mybir.dt.float32r
BF = mybir.dt.bfloat16
AF = mybir.ActivationFunctionType
BF_NP = ml_dtypes.bfloat16


# ---------------------------------------------------------------------------
# BIR legalization for the stock walrus in this container: it encodes at most
# ONE sync wait per instruction, so spill extra waits onto single-wait
# EventSemaphore nops placed immediately before the instruction on the same
# engine.  Also statically verify no wait precedes (in program order) the
# instructions that produce its semaphore value, which would deadlock.
# ---------------------------------------------------------------------------

def split_multiwaits(bir_bytes: bytes) -> bytes:
    bir = orjson.loads(bir_bytes)
    n = [0]
    for fn in bir.get("functions", []):
        for blk in fn.get("blocks", []):
            out = []
            for inst in blk.get("instructions", []):
                si = inst.get("sync_info") or {}
                waits = si.get("on_wait") or []
                if len(waits) > 1:
                    for w in waits[:-1]:
                        n[0] += 1
                        out.append({
                            "debug": inst.get("debug", 0),
                            "engine": inst["engine"],
                            "ins": [], "name": f"WSPLIT-{n[0]}",
                            "opcode": "EventSemaphore", "outs": [],
                            "sync_info": {"on_update": [], "on_wait": [w]},
                        })
                    si["on_wait"] = waits[-1:]
                    inst["sync_info"] = si
                out.append(inst)
            blk["instructions"] = out
    _check_no_forward_waits(bir)
    return orjson.dumps(bir)


def _check_no_forward_waits(bir) -> None:
    issued = {}
    for fn in bir.get("functions", []):
        for blk in fn.get("blocks", []):
            for inst in blk.get("instructions", []):
                si = inst.get("sync_info") or {}
                for w in (si.get("on_wait") or []):
                    if (w.get("sync_type") == "semaphore"
                            and w.get("wait_mode") == "sem-ge-imm"
                            and "barrier" not in (w.get("ant_name") or "")):
                        if issued.get(w["id"], 0) < w["wait_value"]:
                            raise RuntimeError(
                                f"forward wait: {inst['name']} waits "
                                f"{w.get('ant_name')}>={w['wait_value']}")
                for u in (si.get("on_update") or []):
                    if (u.get("sync_type") == "semaphore"
                            and u.get("update_mode") in ("sem-inc", "sem-add-imm")):
                        issued[u["id"]] = issued.get(u["id"], 0) + u.get("update_value", 1)


def _bcast(ap: bass.AP, reps: int) -> bass.AP:
    """Append a stride-0 dim of size `reps` to an AP (free-dim broadcast)."""
    return bass.AP(tensor=ap.tensor, offset=ap.offset,
                   ap=list(ap.ap) + [[0, reps]])


# ---------------------------------------------------------------------------
# Device program (identical on all 8 cores; data differs per core)
# ---------------------------------------------------------------------------

def build_nc(causal=True, stagger=True, ydma_pool=True, pair_t=True) -> bass.Bass:
    nc = bass.Bass("TRN2", target_bir_lowering=False, debug=False)

    xt = nc.dram_tensor("xt", (D_MODEL, T), BF, kind="ExternalInput")
    wq = nc.dram_tensor("wq", (D_MODEL, FG), BF, kind="ExternalInput")
    wk = nc.dram_tensor("wk", (D_MODEL, FG), BF, kind="ExternalInput")
    wv = nc.dram_tensor("wv", (D_MODEL, FG), BF, kind="ExternalInput")
    wo = nc.dram_tensor("wo", (FG, D_MODEL), BF, kind="ExternalInput")
    cvec = nc.dram_tensor("cvec", (T,), F32, kind="ExternalInput")
    mask2d = nc.dram_tensor("mask2d", (128, 256), BF, kind="ExternalInput")
    identd = nc.dram_tensor("identd", (128, 128), BF, kind="ExternalInput")
    y = nc.dram_tensor("y", (T, D_MODEL), F32, kind="ExternalOutput")

    NDT = D_MODEL // 128   # 8 contraction tiles
    NFT = FG // 128        # 4 feature tiles (head pairs)
    NTT = T // 128         # 16 token tiles
    NTS = T // 512         # 4 token strips

    with tile.TileContext(nc) as tc, nc.allow_low_precision(reason="bf16 storage"):
        with (
            tc.tile_pool(name="cst", bufs=1) as cst,
            tc.tile_pool(name="xtp", bufs=1) as xtp,
            tc.tile_pool(name="wp", bufs=1) as wp,
            tc.tile_pool(name="qtp", bufs=1) as qtp,
            tc.tile_pool(name="ktp", bufs=1) as ktp,
            tc.tile_pool(name="otp", bufs=1) as otp,
            tc.tile_pool(name="vpp", bufs=1) as vpp,
            tc.tile_pool(name="stp", bufs=6) as stp,
            tc.tile_pool(name="onp", bufs=2) as onp,
            tc.tile_pool(name="rsp", bufs=4) as rsp,
            tc.tile_pool(name="ysp", bufs=3) as ysp,
            tc.tile_pool(name="psA", bufs=2, space="PSUM") as psA,
            tc.tile_pool(name="psO", bufs=2, space="PSUM") as psO,
            tc.tile_pool(name="psS", bufs=2, space="PSUM") as psS,
        ):
            cb = cst.tile([128, 16], F32, name="cb")
            mask2 = cst.tile([128, 256], BF, name="mask2")
            ident = cst.tile([128, 128], BF, name="ident")

            xts = [xtp.tile([128, T], BF, name=f"xts{i}") for i in range(NDT)]
            wqT = wp.tile([128, NDT * FG], BF, name="wqT")
            wkT = wp.tile([128, NDT * FG], BF, name="wkT")
            wvT = wp.tile([128, NDT * FG], BF, name="wvT")
            woT = wp.tile([128, NFT * D_MODEL], BF, name="woT")
            wq_t = [wqT[:, i * FG:(i + 1) * FG] for i in range(NDT)]
            wk_t = [wkT[:, i * FG:(i + 1) * FG] for i in range(NDT)]
            wv_t = [wvT[:, i * FG:(i + 1) * FG] for i in range(NDT)]
            wot = [woT[:, i * D_MODEL:(i + 1) * D_MODEL] for i in range(NFT)]
            qt = [qtp.tile([128, T], BF, name=f"qt{i}") for i in range(NFT)]
            kt = [ktp.tile([128, T], BF, name=f"kt{i}") for i in range(NFT)]
            ot = [otp.tile([128, T], BF, name=f"ot{i}") for i in range(NFT)]
            vp = [vpp.tile([128, 520], BF, name=f"vp{i}") for i in range(NTT)]

            onorm = {}

            # ---------------- emit helpers --------------------------------

            def emit_kq_strip_ft(wtl, dst, ts, ft):
                ps = psS.tile([128, 512], F32, name="pj", tag="S")
                for d in range(NDT):
                    nc.tensor.matmul(
                        ps[:], wtl[d][:, ft * 128:(ft + 1) * 128],
                        xts[d][:, ts * 512:(ts + 1) * 512],
                        start=(d == 0), stop=(d == NDT - 1))
                nc.vector.tensor_copy(
                    dst[ft][:, ts * 512:(ts + 1) * 512], ps[:])

            def emit_kq_strip(wtl, dst, ts):
                for ft in range(NFT):
                    emit_kq_strip_ft(wtl, dst, ts, ft)

            def emit_v_tile(tt):
                ps = psS.tile([128, 512], F32, name="pv", tag="S")
                for d in range(NDT):
                    nc.tensor.matmul(
                        ps[:], xts[d][:, tt * 128:(tt + 1) * 128], wv_t[d][:],
                        start=(d == 0), stop=(d == NDT - 1))
                vv = vp[tt].rearrange("p (h c) -> p h c", h=8)
                pv = ps.rearrange("p (h c) -> p h c", h=8)
                nc.gpsimd.memset(vv[:, :, 64:65], 1.0)
                nc.vector.tensor_copy(vv[:, :, 0:64], pv[:])

            def emit_y_pair(tt):
                # both output halves of token block tt; one paired DMA row
                ysb = ysp.tile([128, 1024], F32, name="ysb", tag="y")
                for nn in range(2):
                    yp = psS.tile([128, 512], F32, name="yp", tag="S")
                    for ft in range(NFT):
                        nc.tensor.matmul(
                            yp[:], ot[ft][:, tt * 128:(tt + 1) * 128],
                            wot[ft][:, nn * 512:(nn + 1) * 512],
                            start=(ft == 0), stop=(ft == NFT - 1))
                    nc.vector.tensor_copy(ysb[:, nn * 512:(nn + 1) * 512], yp[:])
                if ydma_pool:
                    nc.gpsimd.dma_start(y[tt * 128:(tt + 1) * 128, :], ysb[:])
                else:
                    nc.sync.dma_start(y[tt * 128:(tt + 1) * 128, :], ysb[:])

            def emit_transpose_hp(j, hp):
                on3 = onorm[j].rearrange("p (c f) -> p c f", c=4)
                tp = psS.tile([128, 512], BF, name="tp", tag="S")
                for c in range(4):
                    nc.tensor.transpose(
                        tp[:, c * 128:(c + 1) * 128],
                        on3[:, c, hp * 128:(hp + 1) * 128], ident[:])
                nc.vector.tensor_copy(ot[hp][:, j * 512:(j + 1) * 512], tp[:])

            def emit_attn_strip(j, pacer=None, pair_done=None, need_hook=None):
                nkt = 4 * j + 4 if causal else NTT
                on = onp.tile([128, 2048], BF, name="on", tag="on")
                onorm[j] = on
                on3 = on.rearrange("p (c f) -> p c f", c=4)
                mk = mask2.rearrange("p (h q) -> p h q", h=2)
                opair = {}

                def emit_S(p, ktile):
                    # scores + exp + causal mask for head pair p, key tile kt
                    m = (ktile - 4 * j) if causal else -1
                    lo = 128 * m if m > 0 else 0
                    sp = psA.tile([128, 1024], F32, name="sp", tag="A")
                    se = stp.tile([128, 1024], BF, name="se", tag="se")
                    sp3 = sp.rearrange("p (h q) -> p h q", h=2)
                    se3 = se.rearrange("p (h q) -> p h q", h=2)
                    for hh in range(2):
                        nc.tensor.matmul(
                            sp3[:, hh, lo:512],
                            kt[p][hh * 64:(hh + 1) * 64,
                                  ktile * 128:(ktile + 1) * 128],
                            qt[p][hh * 64:(hh + 1) * 64,
                                  j * 512 + lo:(j + 1) * 512],
                            start=True, stop=True)
                    nc.scalar.activation(
                        se3[:, :, lo:512], sp3[:, :, lo:512], AF.Exp,
                        bias=cb[:, ktile:ktile + 1], scale=0.125)
                    if m >= 0:
                        nc.vector.tensor_mul(
                            se3[:, :, lo:lo + 128],
                            se3[:, :, lo:lo + 128], mk[:])
                    return se3

                def emit_PV(p, ktile, se3):
                    # One accumulation group per head tile: the FIRST matmul
                    # (start=True) zeroes the tile's whole 2KB psum zero
                    # region (all 4 chunk slots); every later matmul
                    # accumulates; the LAST one carries stop.
                    m = (ktile - 4 * j) if causal else -1
                    c0 = m if m > 0 else 0
                    corder = list(range(c0, 4))
                    if 0 <= m < 4 and m in corder and len(corder) > 1:
                        corder = [c for c in corder if c != m] + [m]
                    for hh in range(2):
                        o_ = opair[2 * p + hh]
                        for c in corder:
                            stop = (ktile == nkt - 1 and c == corder[-1])
                            nc.tensor.matmul(
                                o_[:, c * 65:(c + 1) * 65],
                                se3[:, hh, c * 128:(c + 1) * 128],
                                vp[ktile][:, (2 * p + hh) * 65:
                                          (2 * p + hh + 1) * 65],
                                start=(ktile == 0 and c == corder[0]),
                                stop=stop,
                                skip_group_check=True)

                def emit_norm(p):
                    # normalize both heads: O_norm = O[:, c, 0:64] * (1/s_c)
                    for hh in range(2):
                        o_ = opair.pop(2 * p + hh)
                        o3 = o_.rearrange("p (c u) -> p c u", c=4)
                        rs = rsp.tile([128, 4], F32, name="rs", tag="rs")
                        nc.vector.reciprocal(
                            rs.rearrange("p (c u) -> p c u", c=4)[:, :, 0:1],
                            o3[:, :, 64:65])
                        nc.vector.tensor_mul(
                            on3[:, :, (2 * p + hh) * 64:(2 * p + hh + 1) * 64],
                            o3[:, :, 0:64],
                            _bcast(rs[:], 64))

                # software-pipelined over (pair, ktile): scores for item i+1
                # are emitted before the PV of item i, so the PE always has a
                # score matmul in flight while ACT drains the previous tile.
                # `pacer(act_ns, pe_ns)` is called after each item so the
                # scheduler can slot filler PE work into ACT-bound windows.
                items = [(p, ktile) for p in range(4) for ktile in range(nkt)]
                if need_hook is not None:
                    need_hook((j, 2))
                opair[0] = psO.tile([128, 260], F32, name="oA", tag="O")
                opair[1] = psO.tile([128, 260], F32, name="oB", tag="O")
                pend = {0: emit_S(*items[0])}
                for i, (p, ktile) in enumerate(items):
                    if need_hook is not None:
                        need_hook((j, i + 2))
                    nxt = items[i + 1] if i + 1 < len(items) else None
                    if nxt is not None:
                        np_, nk = nxt
                        if nk == 0:
                            opair[2 * np_] = psO.tile([128, 260], F32,
                                                      name="oA", tag="O")
                            opair[2 * np_ + 1] = psO.tile([128, 260], F32,
                                                          name="oB", tag="O")
                        pend[i + 1] = emit_S(np_, nk)
                    emit_PV(p, ktile, pend.pop(i))
                    if ktile == nkt - 1:
                        emit_norm(p)
                        if pair_done is not None:
                            pair_done(j, p)
                    if pacer is not None:
                        m = (ktile - 4 * j) if causal else -1
                        lo = 128 * m if m > 0 else 0
                        c0 = m if m > 0 else 0
                        act_ns = 0.833 * 2 * (512 - lo) + 185
                        pe_ns = 0.4167 * (2 * (512 - lo) + 130 * (4 - c0))
                        pacer(act_ns, pe_ns)

            # ---------------- schedule ------------------------------------

            if not causal:
                # simple sequential schedule (full projections, then attn)
                nc.sync.dma_start(wkT.rearrange("p (d f) -> p d f", d=NDT),
                              wk.rearrange("(d p) f -> p d f", p=128))
                for d in range(NDT):
                    nc.sync.dma_start(xts[d][:], xt[d * 128:(d + 1) * 128, :])
                nc.sync.dma_start(wqT.rearrange("p (d f) -> p d f", d=NDT),
                              wq.rearrange("(d p) f -> p d f", p=128))
                nc.sync.dma_start(wvT.rearrange("p (d f) -> p d f", d=NDT),
                              wv.rearrange("(d p) f -> p d f", p=128))
                nc.sync.dma_start(cb[:], cvec.rearrange("(a b) -> b a", b=128))
                nc.sync.dma_start(mask2[:], mask2d[:])
                nc.sync.dma_start(ident[:], identd[:])
                nc.sync.dma_start(woT.rearrange("p (d f) -> p d f", d=NFT),
                              wo.rearrange("(d p) f -> p d f", p=128))
                for ts in range(NTS):
                    emit_kq_strip(wk_t, kt, ts)
                    emit_kq_strip(wq_t, qt, ts)
                for tt in range(NTT):
                    emit_v_tile(tt)
                for j in range(NTS):
                    emit_attn_strip(
                        j, pair_done=lambda jj, p: emit_transpose_hp(jj, p))
                    for tt in range(4 * j, 4 * j + 4):
                        emit_y_pair(tt)
                _orig0 = nc.to_json_bytes
                nc.to_json_bytes = lambda: split_multiwaits(_orig0())
                return nc

            # ---- paced filler queue: small PE work items (one psS psum
            # tile + one DVE copy each) slotted into ACT-bound windows so
            # the in-order PE never queues more drain than psS can absorb.
            queue = []          # entries: [cost_ns, need_by_strip, closure]
            debt = [0.0]

            def q_push(cost, need, fn):
                queue.append([cost, need, fn])

            def q_pop_one():
                cost, _, fn = queue.pop(0)
                fn()
                return cost

            def pacer(act_ns, pe_ns):
                debt[0] += act_ns - pe_ns
                while queue and debt[0] >= 0.5 * queue[0][0]:
                    debt[0] -= q_pop_one()

            def q_need(key):
                # force-emit entries required at or before `key`, preserving
                # FIFO order among the popped entries
                if not any(e[1] <= key for e in queue):
                    return
                rest = []
                for e in queue:
                    if e[1] <= key:
                        e[2]()
                    else:
                        rest.append(e)
                queue[:] = rest

            def pair_done(j, p):
                # transposes must land before strip j+2 reuses the onorm slot
                q_push(320, (j + 2, 0), lambda j=j, p=p: emit_transpose_hp(j, p))

            def f_kq(wtl, dst, ts, ft, need):
                q_push(1880, need,
                       lambda: emit_kq_strip_ft(wtl, dst, ts, ft))

            def f_v(tt, need):
                q_push(1880, need, lambda tt=tt: emit_v_tile(tt))

            def f_y(tt):
                # two halves as separate fillers sharing one staged row
                st = {}

                def half(nn):
                    if nn == 0:
                        st["ysb"] = ysp.tile([128, 1024], F32, name="ysb",
                                             tag="y")
                    yp = psS.tile([128, 512], F32, name="yp", tag="S")
                    for ft in range(NFT):
                        nc.tensor.matmul(
                            yp[:], ot[ft][:, tt * 128:(tt + 1) * 128],
                            wot[ft][:, nn * 512:(nn + 1) * 512],
                            start=(ft == 0), stop=(ft == NFT - 1))
                    nc.vector.tensor_copy(
                        st["ysb"][:, nn * 512:(nn + 1) * 512], yp[:])
                    if nn == 1:
                        if ydma_pool:
                            nc.gpsimd.dma_start(
                                y[tt * 128:(tt + 1) * 128, :], st["ysb"][:])
                        else:
                            nc.sync.dma_start(
                                y[tt * 128:(tt + 1) * 128, :], st["ysb"][:])
                q_push(1000, (99, 0), lambda: half(0))
                q_push(1000, (99, 0), lambda: half(1))

            # ---- PE warmup: matmuls on uninitialized data at t=0 burn
            # the pstate ramp (and hw-decode warmup) before real work lands
            wsb = cst.tile([128, 512], BF, name="wsb")
            nc.gpsimd.memset(wsb[:], 0.0)
            wps = psS.tile([128, 512], F32, name="wps", tag="S")
            nc.tensor.matmul(wps[:], wsb[:, 0:128], wsb[:],
                             start=True, stop=True)

            # ---- startup: one DMA per weight matrix, xt strip 0 in
            # per-chunk DMAs (K-proj consumes them d by d), strips 1-3 as
            # one wide DMA per row block. Weights ride the SP queue; xt
            # rides the ACT queue so the issue overheads overlap.
            wk3 = wk.rearrange("(d p) f -> p d f", p=128)
            wkT3 = wkT.rearrange("p (d f) -> p d f", d=NDT)
            nc.sync.dma_start(wkT3[:, :, 0:128], wk3[:, :, 0:128])
            for d in range(NDT):
                nc.scalar.dma_start(xts[d][:, 0:512],
                                    xt[d * 128:(d + 1) * 128, 0:512])
            nc.sync.dma_start(wkT3[:, :, 128:512], wk3[:, :, 128:512])
            nc.sync.dma_start(wqT.rearrange("p (d f) -> p d f", d=NDT),
                              wq.rearrange("(d p) f -> p d f", p=128))
            nc.sync.dma_start(wvT.rearrange("p (d f) -> p d f", d=NDT),
                              wv.rearrange("(d p) f -> p d f", p=128))
            nc.sync.dma_start(cb[:], cvec.rearrange("(a b) -> b a", b=128))
            nc.sync.dma_start(mask2[:], mask2d[:])
            nc.sync.dma_start(ident[:], identd[:])
            # pre-load the ACT exp table while the softmax phase is far away
            warm = rsp.tile([128, 4], F32, name="warm", tag="warm")
            nc.scalar.activation(warm[:], cb[:, 0:4], AF.Exp)
            for d in range(NDT):
                nc.scalar.dma_start(xts[d][:, 512:2048],
                                    xt[d * 128:(d + 1) * 128, 512:2048])
            nc.sync.dma_start(woT.rearrange("p (d f) -> p d f", d=NFT),
                              wo.rearrange("(d p) f -> p d f", p=128))

            # pre-attention projections: K strip 0, Q strip 0, V tiles 0-3
            emit_kq_strip(wk_t, kt, 0)
            emit_kq_strip(wq_t, qt, 0)
            for tt in range(0, 4):
                emit_v_tile(tt)

            # filler backlog with just-in-time need keys (strip, item):
            # Q strip-s tile f is first read at item (p=f, kt=0); K strip-s
            # tile f at (p=f, kt=4s); V tile tt at (p=0, kt=tt).
            for s in range(1, NTS):
                nk = 4 * s + 4
                f_kq(wq_t, qt, s, 0, (s, 0))
                f_kq(wk_t, kt, s, 0, (s, 4 * s))
                for i, tt in enumerate(range(4 * s, 4 * s + 4)):
                    f_v(tt, (s, 4 * s + i))
                for f in range(1, NFT):
                    f_kq(wq_t, qt, s, f, (s, f * nk))
                    f_kq(wk_t, kt, s, f, (s, f * nk + 4 * s))

            emit_attn_strip(0, pacer, pair_done, q_need)
            for tt in range(0, 4):
                f_y(tt)
            q_need((1, 2))
            emit_attn_strip(1, pacer, pair_done, q_need)
            for tt in range(4, 8):
                f_y(tt)
            q_need((2, 2))
            emit_attn_strip(2, pacer, pair_done, q_need)
            for tt in range(8, 12):
                f_y(tt)
            q_need((3, 2))
            emit_attn_strip(3, pacer, pair_done, q_need)
            for tt in range(12, 16):
                f_y(tt)
            while queue:
                q_pop_one()

    _orig = nc.to_json_bytes
    nc.to_json_bytes = lambda: split_multiwaits(_orig())
    return nc


_NC = {}


def _get_nc(causal=True) -> bass.Bass:
    if causal not in _NC:
        _NC[causal] = build_nc(causal=causal)
    return _NC[causal]


# ---------------------------------------------------------------------------
# Host-side sharding + gather
# ---------------------------------------------------------------------------

def _kernel_numpy(q, mask, Wq, bq, Wk, bk, Wv, bv, Wo, bo):
    """Exact host fallback for unexpected shapes or arbitrary masks."""
    b, t, d = q.shape
    h = N_HEADS if d == D_MODEL else max(1, d // D_K)
    dk = d // h
    qh = (q @ Wq.T + bq).reshape(b, t, h, dk).transpose(0, 2, 1, 3)
    kh = (q @ Wk.T + bk).reshape(b, t, h, dk).transpose(0, 2, 1, 3)
    vh = (q @ Wv.T + bv).reshape(b, t, h, dk).transpose(0, 2, 1, 3)
    s = np.einsum("bhqd,bhkd->bhqk", qh, kh) / np.sqrt(dk).astype(np.float32)
    s = np.where(mask, -np.inf, s)
    s = s - s.max(axis=-1, keepdims=True)
    p = np.exp(s)
    p /= p.sum(axis=-1, keepdims=True)
    o = np.einsum("bhqk,bhkd->bhqd", p, vh)
    o = o.transpose(0, 2, 1, 3).reshape(b, t, d)
    return (o @ Wo.T + bo).astype(np.float32)


_CAUSAL_REF = None


def _mask_kind(mask):
    """Classify the (B,1,T,T) bool mask: "causal" / "none" / "other"."""
    global _CAUSAL_REF
    mask = np.asarray(mask)
    if mask.shape != (B, 1, T, T):
        return "other"
    if not mask.any():
        return "none"
    if _CAUSAL_REF is None:
        _CAUSAL_REF = np.triu(np.ones((T, T), dtype=bool), k=1)
    for i in range(mask.shape[0]):
        if not np.array_equal(mask[i, 0], _CAUSAL_REF):
            return "other"
    return "causal"


def build_in_maps(q, Wq, bq, Wk, bk, Wv, Wo):
    # causal staircase: M[k, u] = 1 iff k <= u, duplicated for head pairs
    kk = np.arange(128)[:, None]
    uu = np.arange(128)[None, :]
    stair = (kk <= uu).astype(BF_NP)
    mask2 = np.concatenate([stair, stair], axis=1)
    ident = np.eye(128, dtype=BF_NP)

    in_maps = []
    for b in range(B):
        x = np.ascontiguousarray(q[b])                    # (T, D)
        xT = np.ascontiguousarray(x.T.astype(BF_NP))      # (D, T) bf16
        # per-key exp bias: bq . K_k term (softmax-relevant) minus shift M0
        cvec = ((x @ (Wk.T @ bq) + float(bq @ bk)) * 0.125 - M0).astype(np.float32)
        for g in range(2):
            sl = slice(g * FG, (g + 1) * FG)
            in_maps.append({
                "xt": xT,
                "wq": np.ascontiguousarray(Wq[sl].T.astype(BF_NP)),  # (D, FG)
                "wk": np.ascontiguousarray(Wk[sl].T.astype(BF_NP)),
                "wv": np.ascontiguousarray(Wv[sl].T.astype(BF_NP)),
                "wo": np.ascontiguousarray(Wo[:, sl].T.astype(BF_NP)),  # (FG, D)
                "cvec": cvec,
                "mask2d": mask2,
                "identd": ident,
            })
    return in_maps


def kernel(q, mask, Wq, bq, Wk, bk, Wv, bv, Wo, bo):
    q = np.asarray(q, np.float32)
    Wq = np.asarray(Wq, np.float32); bq = np.asarray(bq, np.float32)
    Wk = np.asarray(Wk, np.float32); bk = np.asarray(bk, np.float32)
    Wv = np.asarray(Wv, np.float32); bv = np.asarray(bv, np.float32)
    Wo = np.asarray(Wo, np.float32); bo = np.asarray(bo, np.float32)

    kind = _mask_kind(mask)
    if q.shape != (B, T, D_MODEL) or Wq.shape != (D_MODEL, D_MODEL) or kind == "other":
        return _kernel_numpy(np.asarray(q, np.float32), np.asarray(mask, bool),
                             Wq, bq, Wk, bk, Wv, bv, Wo, bo)

    in_maps = build_in_maps(q, Wq, bq, Wk, bk, Wv, Wo)
    nc = _get_nc(causal=(kind == "causal"))
    res = bass_utils.run_bass_kernel_spmd(nc, in_maps, core_ids=list(range(N_CORES)))

    out = np.empty((B, T, D_MODEL), np.float32)
    # exact bias correction terms (zero when biases are zero):
    # V-bias contributes bv @ Wo.T (softmax rows sum to 1); plus bo.
    corr = (bv @ Wo.T + bo).astype(np.float32)
    for b in range(B):
        out[b] = res.results[2 * b]["y"] + res.results[2 * b + 1]["y"] + corr
    return out
